# revision 28
# baseline (speedup 1.0000x reference)
"""Conditional BatchNorm1d (training-mode, per-class stats) on 8 Trainium2
NeuronCores.

Problem: x [512, 128, 1024] f32, labels [512] i32 in [0,8), weight/bias
[8, 128] f32.  Per-class biased mean/var over the class's (batch, length)
elements per feature, then per-class affine:
    y = x * (rsqrt(var+eps)*w)[lbl] + (b - mean*rsqrt(var+eps)*w)[lbl]

Sharding: data-parallel over batch B across the 8 cores (64 batches each).
Each core streams its x shard once to accumulate per-(class, feature)
sum / sum-of-squares, the tiny [16, 128] partials are AllReduced on-device,
scale/shift are computed and gathered per batch with small matmuls against
the one-hot label mask, and a second streaming pass applies the affine.

Layout: the host hands each core its shard transposed to feature-major
[F=128, B_LOC=64, L=1024] so a single DMA can move a 2-batch group with
8 KiB of DRAM-contiguous data per partition (measured ~390 GB/s vs
~360 GB/s for per-batch 4 KiB lines).  The tail RES batches of pass 1 stay
resident in SBUF, so pass 2 only re-reads the first B_LOC-RES batches.
"""

import sys

if "/opt/trn_rl_repo" not in sys.path:
    sys.path.insert(0, "/opt/trn_rl_repo")

import numpy as np

import concourse.bacc as bacc
import concourse.tile as tile
from concourse import mybir
from concourse import bass_utils

B, F, L = 512, 128, 1024
K = 8
N_CORES = 8
B_LOC = B // N_CORES  # 64
EPS = 1e-5
GRP = 2               # batches per DMA group
RES = 20              # resident batches (multiple of GRP)

F32 = mybir.dt.float32
AFT = mybir.ActivationFunctionType

_built = None


def _build():
    nc = bacc.Bacc("TRN2", target_bir_lowering=False, debug=False,
                   num_devices=N_CORES)

    x = nc.dram_tensor("x", [F, B_LOC, L], F32, kind="ExternalInput")
    # One-hot label mask, transposed: maskT[k, j] = 1 iff labels[shard j] == k
    maskT = nc.dram_tensor("maskT", [K, B_LOC], F32, kind="ExternalInput")
    # Block-diagonal mask for the stats matmul: mask2[j, k] = maskT[k, j] and
    # mask2[64+j, 8+k] = maskT[k, j] (sum half / sum-of-squares half).
    mask2 = nc.dram_tensor("mask2", [2 * B_LOC, 2 * K], F32,
                           kind="ExternalInput")
    ident = nc.dram_tensor("ident", [128, 128], F32, kind="ExternalInput")
    rcp_cnt = nc.dram_tensor("rcp_cnt", [K, 1], F32, kind="ExternalInput")
    epsv = nc.dram_tensor("epsv", [K, 1], F32, kind="ExternalInput")
    weight = nc.dram_tensor("weight", [K, F], F32, kind="ExternalInput")
    bias = nc.dram_tensor("bias", [K, F], F32, kind="ExternalInput")
    y = nc.dram_tensor("y", [F, B_LOC, L], F32, kind="ExternalOutput")

    n_grp = B_LOC // GRP
    res_grp = RES // GRP
    stream_grp = n_grp - res_grp   # groups re-read in pass 2

    with tile.TileContext(nc) as tc:
        with (
            tc.tile_pool(name="const", bufs=1) as constp,
            tc.tile_pool(name="xin", bufs=5) as xin,
            tc.tile_pool(name="xres", bufs=res_grp) as xres,
            tc.tile_pool(name="stats", bufs=1) as statsp,
            tc.tile_pool(name="psum", bufs=1, space="PSUM") as psum,
            tc.tile_pool(name="dram", bufs=1, space="DRAM") as dram,
            tc.tile_pool(name="xin2", bufs=6) as xin2,
            tc.tile_pool(name="yout", bufs=3) as yout,
        ):
            # const loads issue from the ACT sequencer so the first x loads
            # lead the in-order Sync stream.  Consts are packed into two
            # tiles: every tile burns a 4KB/partition slot regardless of
            # size, so separate tiny tiles would waste ~24KB/partition.
            cpack1 = constp.tile([128, 144], F32)
            identt = cpack1[:, 0:128]
            mask2t = cpack1[:, 128:144]
            nc.scalar.dma_start(identt, ident[:])
            nc.scalar.dma_start(mask2t, mask2[:])
            cpack2 = constp.tile([K, 322], F32)
            maskTt = cpack2[:, 0:B_LOC]
            rcpt = cpack2[:, B_LOC:B_LOC + 1]
            epst = cpack2[:, B_LOC + 1:B_LOC + 2]
            wt = cpack2[:, 66:194]
            bt = cpack2[:, 194:322]
            nc.scalar.dma_start(maskTt, maskT[:])
            nc.scalar.dma_start(rcpt, rcp_cnt[:])
            nc.scalar.dma_start(epst, epsv[:])
            nc.scalar.dma_start(wt, weight[:])
            nc.scalar.dma_start(bt, bias[:])

            # ---- pass 1: per-batch row sums / sums of squares ----
            # S[:, b] = sum_l x[:, b, l] (DVE); Q[:, b] = sum_l x[:, b, l]^2
            # (ACT).  Separate S/Q tiles: a shared tile would make Tile
            # serialize the two engines on false WAW sharing.
            # Packed stat tiles (slot economy); S and Q stay in separate
            # tiles so ACT and DVE never false-share a written tile in the
            # hot loop.  spackA is all-DVE-written, spackB all-ACT + late
            # DVE, spackC holds the small serial chain.
            spackA = statsp.tile([128, 256], F32)
            S = spackA[:, 0:B_LOC]
            sqt = spackA[:, 64:192]
            ssel = spackA[:, 192:256]
            spackB = statsp.tile([128, 128], F32)
            Q = spackB[:, 0:B_LOC]
            tsel = spackB[:, 64:128]
            spackC = statsp.tile([2 * K, 1280], F32)
            part = spackC[:, 0:128]
            Gs = spackC[0:K, 128:256]
            Gq = spackC[0:K, 256:384]
            chain = spackC[0:K, 384:1280]
            # ACT square scratch lives in PSUM (2 banks), saving SBUF
            scratch_a = psum.tile([128, L], F32)
            res_tiles = {}
            for g in range(n_grp):
                if g >= stream_grp:
                    xt = xres.tile([F, GRP * L], F32)
                    res_tiles[g] = xt
                else:
                    xt = xin.tile([F, GRP * L], F32, tag="xs")
                nc.sync.dma_start(xt[:], x[:, g * GRP:(g + 1) * GRP, :])
                for i in range(GRP):
                    b = g * GRP + i
                    xs = xt[:, i * L:(i + 1) * L]
                    nc.scalar.activation(scratch_a[:], xs, AFT.Square,
                                         accum_out=Q[:, b:b + 1])
                    nc.vector.reduce_sum(S[:, b:b + 1], xs,
                                         axis=mybir.AxisListType.X)

            # ---- per-class reduction: transpose + masked matmul ----
            # sqt partitions 0..63 = S^T (batch-major), 64..127 = Q^T.
            st_ps = psum.tile([B_LOC, 128], F32)
            nc.tensor.transpose(st_ps[:], S, identt)
            qt_ps = psum.tile([B_LOC, 128], F32)
            nc.tensor.transpose(qt_ps[:], Q, identt)
            nc.vector.tensor_copy(spackA[0:B_LOC, 64:192], st_ps[:])
            nc.vector.tensor_copy(spackA[B_LOC:128, 64:192], qt_ps[:])

            part_ps = psum.tile([2 * K, 128], F32)
            nc.tensor.matmul(part_ps[:], mask2t, sqt, start=True,
                             stop=True)
            nc.vector.tensor_copy(part, part_ps[:])

            # ---- all-reduce the [16, 128] partials across the 8 cores ----
            cc_in = dram.tile([2 * K, 128], F32)
            cc_out = dram.tile([2 * K, 128], F32)
            nc.sync.dma_start(cc_in[:], part)
            nc.gpsimd.collective_compute(
                "AllReduce",
                mybir.AluOpType.add,
                replica_groups=[list(range(N_CORES))],
                ins=[cc_in.opt()],
                outs=[cc_out.opt()],
            )
            # G loads issue from the ACT sequencer: they must wait for the
            # AllReduce, and a wait on the in-order Sync stream would block
            # the pass-2 prefetch issues queued behind it.
            nc.scalar.dma_start(Gs, cc_out[0:K])
            nc.scalar.dma_start(Gq, cc_out[K:2 * K])

            # ---- scale/shift per (class, feature) ----
            mean = chain[:, 0 * F:1 * F]
            msq = chain[:, 1 * F:2 * F]
            var = chain[:, 2 * F:3 * F]
            std = chain[:, 3 * F:4 * F]
            inv = chain[:, 4 * F:5 * F]
            scal = chain[:, 5 * F:6 * F]
            shft = chain[:, 6 * F:7 * F]
            nc.vector.tensor_scalar_mul(mean, Gs, rcpt)
            nc.vector.tensor_scalar_mul(msq, Gq, rcpt)
            nc.vector.tensor_mul(var, mean, mean)
            nc.vector.tensor_sub(var, msq, var)
            nc.scalar.activation(std, var, AFT.Sqrt, bias=epst)
            nc.vector.reciprocal(inv, std)
            nc.vector.tensor_mul(scal, inv, wt)
            nc.vector.tensor_mul(shft, mean, scal)
            nc.vector.tensor_sub(shft, bt, shft)

            # ---- select per-batch scale/shift columns: [F, B_LOC] ----
            ssel_ps = psum.tile([F, B_LOC], F32)
            nc.tensor.matmul(ssel_ps[:], scal, maskTt, start=True,
                             stop=True)
            nc.vector.tensor_copy(ssel, ssel_ps[:])
            tsel_ps = psum.tile([F, B_LOC], F32)
            nc.tensor.matmul(tsel_ps[:], shft, maskTt, start=True,
                             stop=True)
            nc.vector.tensor_copy(tsel, tsel_ps[:])

            # ---- pass 2: y[:, b] = x[:, b] * ssel[:, b] + tsel[:, b] ----
            # Whole group handled by one engine (group parity): keeps the
            # ACT/DVE streams independent, no shared-tile serialization.
            # Resident groups first: their applies are ready the moment
            # ssel/tsel land, keeping stores busy while reloads stream.
            def apply_group(g, xt_tile):
                yt = yout.tile([F, GRP * L], F32)
                for i in range(GRP):
                    b = g * GRP + i
                    xs = xt_tile[:, i * L:(i + 1) * L]
                    ys = yt[:, i * L:(i + 1) * L]
                    if g % 2 == 0:
                        nc.scalar.activation(ys, xs, AFT.Identity,
                                             bias=tsel[:, b:b + 1],
                                             scale=ssel[:, b:b + 1])
                    else:
                        nc.vector.tensor_scalar(ys, xs,
                                                ssel[:, b:b + 1],
                                                tsel[:, b:b + 1],
                                                mybir.AluOpType.mult,
                                                mybir.AluOpType.add)
                nc.gpsimd.dma_start(y[:, g * GRP:(g + 1) * GRP, :], yt[:])

            for g in range(stream_grp, n_grp):
                apply_group(g, res_tiles[g])
            for g in range(stream_grp):
                # First reloads reuse the freed pass-1 xin slots: deeper
                # prefetch over the AllReduce window at no SBUF cost.
                pool = xin if g < 5 else xin2
                xt2 = pool.tile([F, GRP * L], F32, tag="xs")
                nc.sync.dma_start(xt2[:], x[:, g * GRP:(g + 1) * GRP, :])
                apply_group(g, xt2)

    nc.finalize()
    return nc


def _get_nc():
    global _built
    if _built is None:
        _built = _build()
    return _built


def _host_inputs(x, labels, weight, bias):
    labels = np.asarray(labels).astype(np.int64)
    counts = np.bincount(labels, minlength=K).astype(np.float64) * L
    rcp = (1.0 / np.maximum(counts, 1.0)).astype(np.float32).reshape(K, 1)
    ident = np.eye(128, dtype=np.float32)

    in_maps = []
    for c in range(N_CORES):
        lab = labels[c * B_LOC:(c + 1) * B_LOC]
        maskT = np.zeros((K, B_LOC), dtype=np.float32)
        maskT[lab, np.arange(B_LOC)] = 1.0
        mask2 = np.zeros((2 * B_LOC, 2 * K), dtype=np.float32)
        mask2[:B_LOC, :K] = maskT.T
        mask2[B_LOC:, K:] = maskT.T
        in_maps.append({
            # feature-major shard: [F, B_LOC, L]
            "x": np.ascontiguousarray(
                x[c * B_LOC:(c + 1) * B_LOC].transpose(1, 0, 2)),
            "maskT": maskT,
            "mask2": mask2,
            "ident": ident,
            "rcp_cnt": rcp,
            "epsv": np.full((K, 1), EPS, dtype=np.float32),
            "weight": np.ascontiguousarray(weight.astype(np.float32)),
            "bias": np.ascontiguousarray(bias.astype(np.float32)),
        })
    return in_maps


def run(x, labels, weight, bias, trace=False):
    nc = _get_nc()
    in_maps = _host_inputs(x, labels, weight, bias)
    res = bass_utils.run_bass_kernel_spmd(nc, in_maps, list(range(N_CORES)),
                                          trace=trace)
    out = np.concatenate(
        [res.results[c]["y"].transpose(1, 0, 2) for c in range(N_CORES)],
        axis=0)
    return out, res


def kernel(x, labels, weight, bias):
    out, _ = run(np.asarray(x, dtype=np.float32), labels,
                 np.asarray(weight, dtype=np.float32),
                 np.asarray(bias, dtype=np.float32))
    return out


# revision 29
# speedup vs baseline: 1.0171x; 1.0171x over previous
"""Conditional BatchNorm1d (training-mode, per-class stats) on 8 Trainium2
NeuronCores.

Problem: x [512, 128, 1024] f32, labels [512] i32 in [0,8), weight/bias
[8, 128] f32.  Per-class biased mean/var over the class's (batch, length)
elements per feature, then per-class affine:
    y = x * (rsqrt(var+eps)*w)[lbl] + (b - mean*rsqrt(var+eps)*w)[lbl]

Sharding: data-parallel over batch B across the 8 cores (64 batches each).
Each core streams its x shard once to accumulate per-(class, feature)
sum / sum-of-squares, the tiny [16, 128] partials are AllReduced on-device,
scale/shift are computed and gathered per batch with small matmuls against
the one-hot label mask, and a second streaming pass applies the affine.

Layout: the host hands each core its shard transposed to feature-major
[F=128, B_LOC=64, L=1024] so a single DMA can move a 2-batch group with
8 KiB of DRAM-contiguous data per partition (measured ~390 GB/s vs
~360 GB/s for per-batch 4 KiB lines).  The tail RES batches of pass 1 stay
resident in SBUF, so pass 2 only re-reads the first B_LOC-RES batches.
"""

import sys

if "/opt/trn_rl_repo" not in sys.path:
    sys.path.insert(0, "/opt/trn_rl_repo")

import numpy as np

import concourse.bacc as bacc
import concourse.tile as tile
from concourse import mybir
from concourse import bass_utils

B, F, L = 512, 128, 1024
K = 8
N_CORES = 8
B_LOC = B // N_CORES  # 64
EPS = 1e-5
GRP = 2               # batches per DMA group
RES = 20              # resident batches (multiple of GRP)

F32 = mybir.dt.float32
AFT = mybir.ActivationFunctionType

_built = None


def _build():
    nc = bacc.Bacc("TRN2", target_bir_lowering=False, debug=False,
                   num_devices=N_CORES)

    x = nc.dram_tensor("x", [F, B_LOC, L], F32, kind="ExternalInput")
    # One-hot label mask, transposed: maskT[k, j] = 1 iff labels[shard j] == k
    maskT = nc.dram_tensor("maskT", [K, B_LOC], F32, kind="ExternalInput")
    # Block-diagonal mask for the stats matmul: mask2[j, k] = maskT[k, j] and
    # mask2[64+j, 8+k] = maskT[k, j] (sum half / sum-of-squares half).
    mask2 = nc.dram_tensor("mask2", [2 * B_LOC, 2 * K], F32,
                           kind="ExternalInput")
    ident = nc.dram_tensor("ident", [128, 128], F32, kind="ExternalInput")
    rcp_cnt = nc.dram_tensor("rcp_cnt", [K, 1], F32, kind="ExternalInput")
    epsv = nc.dram_tensor("epsv", [K, 1], F32, kind="ExternalInput")
    weight = nc.dram_tensor("weight", [K, F], F32, kind="ExternalInput")
    bias = nc.dram_tensor("bias", [K, F], F32, kind="ExternalInput")
    y = nc.dram_tensor("y", [F, B_LOC, L], F32, kind="ExternalOutput")

    n_grp = B_LOC // GRP
    res_grp = RES // GRP
    stream_grp = n_grp - res_grp   # groups re-read in pass 2

    with tile.TileContext(nc) as tc:
        with (
            tc.tile_pool(name="const", bufs=1) as constp,
            tc.tile_pool(name="xin", bufs=5) as xin,
            tc.tile_pool(name="xres", bufs=res_grp) as xres,
            tc.tile_pool(name="stats", bufs=1) as statsp,
            tc.tile_pool(name="psum", bufs=1, space="PSUM") as psum,
            tc.tile_pool(name="dram", bufs=1, space="DRAM") as dram,
            tc.tile_pool(name="xin2", bufs=6) as xin2,
            tc.tile_pool(name="yout", bufs=3) as yout,
        ):
            # const loads issue from the ACT sequencer so the first x loads
            # lead the in-order Sync stream.  Consts are packed into two
            # tiles: every tile burns a 4KB/partition slot regardless of
            # size, so separate tiny tiles would waste ~24KB/partition.
            cpack1 = constp.tile([128, 144], F32)
            identt = cpack1[:, 0:128]
            mask2t = cpack1[:, 128:144]
            nc.scalar.dma_start(identt, ident[:])
            nc.scalar.dma_start(mask2t, mask2[:])
            cpack2 = constp.tile([K, 322], F32)
            maskTt = cpack2[:, 0:B_LOC]
            rcpt = cpack2[:, B_LOC:B_LOC + 1]
            epst = cpack2[:, B_LOC + 1:B_LOC + 2]
            wt = cpack2[:, 66:194]
            bt = cpack2[:, 194:322]
            nc.scalar.dma_start(maskTt, maskT[:])
            nc.scalar.dma_start(rcpt, rcp_cnt[:])
            nc.scalar.dma_start(epst, epsv[:])
            nc.scalar.dma_start(wt, weight[:])
            nc.scalar.dma_start(bt, bias[:])

            # ---- pass 1: per-batch row sums / sums of squares ----
            # S[:, b] = sum_l x[:, b, l] (DVE); Q[:, b] = sum_l x[:, b, l]^2
            # (ACT).  Separate S/Q tiles: a shared tile would make Tile
            # serialize the two engines on false WAW sharing.
            # Packed stat tiles (slot economy); S and Q stay in separate
            # tiles so ACT and DVE never false-share a written tile in the
            # hot loop.  spackA is all-DVE-written, spackB all-ACT + late
            # DVE, spackC holds the small serial chain.
            spackA = statsp.tile([128, 256], F32)
            S = spackA[:, 0:B_LOC]
            sqt = spackA[:, 64:192]
            ssel = spackA[:, 192:256]
            spackB = statsp.tile([128, 128], F32)
            Q = spackB[:, 0:B_LOC]
            tsel = spackB[:, 64:128]
            spackC = statsp.tile([2 * K, 1280], F32)
            part = spackC[:, 0:128]
            Gs = spackC[0:K, 128:256]
            Gq = spackC[0:K, 256:384]
            chain = spackC[0:K, 384:1280]
            # ACT square scratch lives in PSUM (2 banks), saving SBUF
            scratch_a = psum.tile([128, L], F32)
            res_tiles = {}
            for g in range(n_grp):
                if g >= stream_grp:
                    xt = xres.tile([F, GRP * L], F32)
                    res_tiles[g] = xt
                else:
                    xt = xin.tile([F, GRP * L], F32, tag="xs")
                nc.sync.dma_start(xt[:], x[:, g * GRP:(g + 1) * GRP, :])
                for i in range(GRP):
                    b = g * GRP + i
                    xs = xt[:, i * L:(i + 1) * L]
                    nc.scalar.activation(scratch_a[:], xs, AFT.Square,
                                         accum_out=Q[:, b:b + 1])
                    nc.vector.reduce_sum(S[:, b:b + 1], xs,
                                         axis=mybir.AxisListType.X)

            # ---- per-class reduction: transpose + masked matmul ----
            # sqt partitions 0..63 = S^T (batch-major), 64..127 = Q^T.
            st_ps = psum.tile([B_LOC, 128], F32)
            nc.tensor.transpose(st_ps[:], S, identt)
            qt_ps = psum.tile([B_LOC, 128], F32)
            nc.tensor.transpose(qt_ps[:], Q, identt)
            nc.vector.tensor_copy(spackA[0:B_LOC, 64:192], st_ps[:])
            nc.vector.tensor_copy(spackA[B_LOC:128, 64:192], qt_ps[:])

            part_ps = psum.tile([2 * K, 128], F32)
            nc.tensor.matmul(part_ps[:], mask2t, sqt, start=True,
                             stop=True)
            nc.vector.tensor_copy(part, part_ps[:])

            # ---- all-reduce the [16, 128] partials across the 8 cores ----
            cc_in = dram.tile([2 * K, 128], F32)
            cc_out = dram.tile([2 * K, 128], F32)
            # upload via GpSimd: it waits on `part`, and a wait on the
            # in-order Sync stream would stall the pass-2 prefetch issues
            nc.gpsimd.dma_start(cc_in[:], part)
            nc.gpsimd.collective_compute(
                "AllReduce",
                mybir.AluOpType.add,
                replica_groups=[list(range(N_CORES))],
                ins=[cc_in.opt()],
                outs=[cc_out.opt()],
            )
            # G loads issue from the ACT sequencer: they must wait for the
            # AllReduce, and a wait on the in-order Sync stream would block
            # the pass-2 prefetch issues queued behind it.
            nc.scalar.dma_start(Gs, cc_out[0:K])
            nc.scalar.dma_start(Gq, cc_out[K:2 * K])

            # ---- scale/shift per (class, feature) ----
            mean = chain[:, 0 * F:1 * F]
            msq = chain[:, 1 * F:2 * F]
            var = chain[:, 2 * F:3 * F]
            std = chain[:, 3 * F:4 * F]
            inv = chain[:, 4 * F:5 * F]
            scal = chain[:, 5 * F:6 * F]
            shft = chain[:, 6 * F:7 * F]
            nc.vector.tensor_scalar_mul(mean, Gs, rcpt)
            nc.vector.tensor_scalar_mul(msq, Gq, rcpt)
            nc.vector.tensor_mul(var, mean, mean)
            nc.vector.tensor_sub(var, msq, var)
            nc.scalar.activation(std, var, AFT.Sqrt, bias=epst)
            nc.vector.reciprocal(inv, std)
            nc.vector.tensor_mul(scal, inv, wt)
            nc.vector.tensor_mul(shft, mean, scal)
            nc.vector.tensor_sub(shft, bt, shft)

            # ---- select per-batch scale/shift columns: [F, B_LOC] ----
            ssel_ps = psum.tile([F, B_LOC], F32)
            nc.tensor.matmul(ssel_ps[:], scal, maskTt, start=True,
                             stop=True)
            nc.vector.tensor_copy(ssel, ssel_ps[:])
            tsel_ps = psum.tile([F, B_LOC], F32)
            nc.tensor.matmul(tsel_ps[:], shft, maskTt, start=True,
                             stop=True)
            nc.vector.tensor_copy(tsel, tsel_ps[:])

            # ---- pass 2: y[:, b] = x[:, b] * ssel[:, b] + tsel[:, b] ----
            # Whole group handled by one engine (group parity): keeps the
            # ACT/DVE streams independent, no shared-tile serialization.
            # Resident groups first: their applies are ready the moment
            # ssel/tsel land, keeping stores busy while reloads stream.
            def apply_group(g, xt_tile):
                yt = yout.tile([F, GRP * L], F32)
                for i in range(GRP):
                    b = g * GRP + i
                    xs = xt_tile[:, i * L:(i + 1) * L]
                    ys = yt[:, i * L:(i + 1) * L]
                    if g % 2 == 0:
                        nc.scalar.activation(ys, xs, AFT.Identity,
                                             bias=tsel[:, b:b + 1],
                                             scale=ssel[:, b:b + 1])
                    else:
                        nc.vector.tensor_scalar(ys, xs,
                                                ssel[:, b:b + 1],
                                                tsel[:, b:b + 1],
                                                mybir.AluOpType.mult,
                                                mybir.AluOpType.add)
                nc.gpsimd.dma_start(y[:, g * GRP:(g + 1) * GRP, :], yt[:])

            # Interleave resident and streamed groups: resident applies are
            # ready the instant ssel/tsel land (stores start immediately),
            # while early streamed applies free load slots so the reload
            # stream never waits behind a block of resident-only work.
            for j in range(max(res_grp, stream_grp)):
                if j < res_grp:
                    apply_group(stream_grp + j, res_tiles[stream_grp + j])
                if j < stream_grp:
                    # First reloads reuse the freed pass-1 xin slots: deeper
                    # prefetch over the AllReduce window at no SBUF cost.
                    pool = xin if j < 5 else xin2
                    xt2 = pool.tile([F, GRP * L], F32, tag="xs")
                    nc.sync.dma_start(xt2[:], x[:, j * GRP:(j + 1) * GRP, :])
                    apply_group(j, xt2)

    nc.finalize()
    return nc


def _get_nc():
    global _built
    if _built is None:
        _built = _build()
    return _built


def _host_inputs(x, labels, weight, bias):
    labels = np.asarray(labels).astype(np.int64)
    counts = np.bincount(labels, minlength=K).astype(np.float64) * L
    rcp = (1.0 / np.maximum(counts, 1.0)).astype(np.float32).reshape(K, 1)
    ident = np.eye(128, dtype=np.float32)

    in_maps = []
    for c in range(N_CORES):
        lab = labels[c * B_LOC:(c + 1) * B_LOC]
        maskT = np.zeros((K, B_LOC), dtype=np.float32)
        maskT[lab, np.arange(B_LOC)] = 1.0
        mask2 = np.zeros((2 * B_LOC, 2 * K), dtype=np.float32)
        mask2[:B_LOC, :K] = maskT.T
        mask2[B_LOC:, K:] = maskT.T
        in_maps.append({
            # feature-major shard: [F, B_LOC, L]
            "x": np.ascontiguousarray(
                x[c * B_LOC:(c + 1) * B_LOC].transpose(1, 0, 2)),
            "maskT": maskT,
            "mask2": mask2,
            "ident": ident,
            "rcp_cnt": rcp,
            "epsv": np.full((K, 1), EPS, dtype=np.float32),
            "weight": np.ascontiguousarray(weight.astype(np.float32)),
            "bias": np.ascontiguousarray(bias.astype(np.float32)),
        })
    return in_maps


def run(x, labels, weight, bias, trace=False):
    nc = _get_nc()
    in_maps = _host_inputs(x, labels, weight, bias)
    res = bass_utils.run_bass_kernel_spmd(nc, in_maps, list(range(N_CORES)),
                                          trace=trace)
    out = np.concatenate(
        [res.results[c]["y"].transpose(1, 0, 2) for c in range(N_CORES)],
        axis=0)
    return out, res


def kernel(x, labels, weight, bias):
    out, _ = run(np.asarray(x, dtype=np.float32), labels,
                 np.asarray(weight, dtype=np.float32),
                 np.asarray(bias, dtype=np.float32))
    return out


# revision 30
# speedup vs baseline: 1.0194x; 1.0023x over previous
"""Conditional BatchNorm1d (training-mode, per-class stats) on 8 Trainium2
NeuronCores.

Problem: x [512, 128, 1024] f32, labels [512] i32 in [0,8), weight/bias
[8, 128] f32.  Per-class biased mean/var over the class's (batch, length)
elements per feature, then per-class affine:
    y = x * (rsqrt(var+eps)*w)[lbl] + (b - mean*rsqrt(var+eps)*w)[lbl]

Sharding: data-parallel over batch B across the 8 cores (64 batches each).
Each core streams its x shard once to accumulate per-(class, feature)
sum / sum-of-squares, the tiny [16, 128] partials are AllReduced on-device,
scale/shift are computed and gathered per batch with small matmuls against
the one-hot label mask, and a second streaming pass applies the affine.

Layout: the host hands each core its shard transposed to feature-major
[F=128, B_LOC=64, L=1024] so a single DMA can move a 2-batch group with
8 KiB of DRAM-contiguous data per partition (measured ~390 GB/s vs
~360 GB/s for per-batch 4 KiB lines).  The tail RES batches of pass 1 stay
resident in SBUF, so pass 2 only re-reads the first B_LOC-RES batches.
"""

import sys

if "/opt/trn_rl_repo" not in sys.path:
    sys.path.insert(0, "/opt/trn_rl_repo")

import numpy as np

import concourse.bacc as bacc
import concourse.tile as tile
from concourse import mybir
from concourse import bass_utils

B, F, L = 512, 128, 1024
K = 8
N_CORES = 8
B_LOC = B // N_CORES  # 64
EPS = 1e-5
GRP = 2               # batches per DMA group
RES = 20              # resident batches (multiple of GRP)

F32 = mybir.dt.float32
AFT = mybir.ActivationFunctionType

_built = None


def _build():
    nc = bacc.Bacc("TRN2", target_bir_lowering=False, debug=False,
                   num_devices=N_CORES)

    x = nc.dram_tensor("x", [F, B_LOC, L], F32, kind="ExternalInput")
    # One-hot label mask, transposed: maskT[k, j] = 1 iff labels[shard j] == k
    maskT = nc.dram_tensor("maskT", [K, B_LOC], F32, kind="ExternalInput")
    # Block-diagonal mask for the stats matmul: mask2[j, k] = maskT[k, j] and
    # mask2[64+j, 8+k] = maskT[k, j] (sum half / sum-of-squares half).
    mask2 = nc.dram_tensor("mask2", [2 * B_LOC, 2 * K], F32,
                           kind="ExternalInput")
    ident = nc.dram_tensor("ident", [128, 128], F32, kind="ExternalInput")
    rcp_cnt = nc.dram_tensor("rcp_cnt", [K, 1], F32, kind="ExternalInput")
    epsv = nc.dram_tensor("epsv", [K, 1], F32, kind="ExternalInput")
    weight = nc.dram_tensor("weight", [K, F], F32, kind="ExternalInput")
    bias = nc.dram_tensor("bias", [K, F], F32, kind="ExternalInput")
    y = nc.dram_tensor("y", [F, B_LOC, L], F32, kind="ExternalOutput")

    n_grp = B_LOC // GRP
    res_grp = RES // GRP
    stream_grp = n_grp - res_grp   # groups re-read in pass 2

    with tile.TileContext(nc) as tc:
        with (
            tc.tile_pool(name="const", bufs=1) as constp,
            tc.tile_pool(name="xin", bufs=5) as xin,
            tc.tile_pool(name="xres", bufs=res_grp) as xres,
            tc.tile_pool(name="stats", bufs=1) as statsp,
            tc.tile_pool(name="psum", bufs=1, space="PSUM") as psum,
            tc.tile_pool(name="dram", bufs=1, space="DRAM") as dram,
            tc.tile_pool(name="xin2", bufs=6) as xin2,
            tc.tile_pool(name="yout", bufs=3) as yout,
        ):
            # const loads issue from the ACT sequencer so the first x loads
            # lead the in-order Sync stream.  Consts are packed into two
            # tiles: every tile burns a 4KB/partition slot regardless of
            # size, so separate tiny tiles would waste ~24KB/partition.
            cpack1 = constp.tile([128, 144], F32)
            identt = cpack1[:, 0:128]
            mask2t = cpack1[:, 128:144]
            nc.scalar.dma_start(identt, ident[:])
            nc.scalar.dma_start(mask2t, mask2[:])
            cpack2 = constp.tile([K, 322], F32)
            maskTt = cpack2[:, 0:B_LOC]
            rcpt = cpack2[:, B_LOC:B_LOC + 1]
            epst = cpack2[:, B_LOC + 1:B_LOC + 2]
            wt = cpack2[:, 66:194]
            bt = cpack2[:, 194:322]
            nc.scalar.dma_start(maskTt, maskT[:])
            nc.scalar.dma_start(rcpt, rcp_cnt[:])
            nc.scalar.dma_start(epst, epsv[:])
            nc.scalar.dma_start(wt, weight[:])
            nc.scalar.dma_start(bt, bias[:])

            # ---- pass 1: per-batch row sums / sums of squares ----
            # S[:, b] = sum_l x[:, b, l] (DVE); Q[:, b] = sum_l x[:, b, l]^2
            # (ACT).  Separate S/Q tiles: a shared tile would make Tile
            # serialize the two engines on false WAW sharing.
            # Packed stat tiles (slot economy); S and Q stay in separate
            # tiles so ACT and DVE never false-share a written tile in the
            # hot loop.  spackA is all-DVE-written, spackB all-ACT + late
            # DVE, spackC holds the small serial chain.
            spackA = statsp.tile([128, 256], F32)
            S = spackA[:, 0:B_LOC]
            sqt = spackA[:, 64:192]
            ssel = spackA[:, 192:256]
            spackB = statsp.tile([128, 128], F32)
            Q = spackB[:, 0:B_LOC]
            tsel = spackB[:, 64:128]
            spackC = statsp.tile([2 * K, 1280], F32)
            part = spackC[:, 0:128]
            Gs = spackC[0:K, 128:256]
            Gq = spackC[0:K, 256:384]
            chain = spackC[0:K, 384:1280]
            # ACT square scratch lives in PSUM (2 banks), saving SBUF
            scratch_a = psum.tile([128, L], F32)
            res_tiles = {}
            for g in range(n_grp):
                if g >= stream_grp:
                    xt = xres.tile([F, GRP * L], F32)
                    res_tiles[g] = xt
                else:
                    xt = xin.tile([F, GRP * L], F32, tag="xs")
                nc.sync.dma_start(xt[:], x[:, g * GRP:(g + 1) * GRP, :])
                for i in range(GRP):
                    b = g * GRP + i
                    xs = xt[:, i * L:(i + 1) * L]
                    nc.scalar.activation(scratch_a[:], xs, AFT.Square,
                                         accum_out=Q[:, b:b + 1])
                    nc.vector.reduce_sum(S[:, b:b + 1], xs,
                                         axis=mybir.AxisListType.X)

            # ---- per-class reduction: transpose + masked matmul ----
            # sqt partitions 0..63 = S^T (batch-major), 64..127 = Q^T.
            st_ps = psum.tile([B_LOC, 128], F32)
            nc.tensor.transpose(st_ps[:], S, identt)
            qt_ps = psum.tile([B_LOC, 128], F32)
            nc.tensor.transpose(qt_ps[:], Q, identt)
            nc.vector.tensor_copy(spackA[0:B_LOC, 64:192], st_ps[:])
            nc.vector.tensor_copy(spackA[B_LOC:128, 64:192], qt_ps[:])

            part_ps = psum.tile([2 * K, 128], F32)
            nc.tensor.matmul(part_ps[:], mask2t, sqt, start=True,
                             stop=True)
            nc.vector.tensor_copy(part, part_ps[:])

            # ---- all-reduce the [16, 128] partials across the 8 cores ----
            cc_in = dram.tile([2 * K, 128], F32)
            cc_out = dram.tile([2 * K, 128], F32)
            # upload via GpSimd: it waits on `part`, and a wait on the
            # in-order Sync stream would stall the pass-2 prefetch issues
            nc.gpsimd.dma_start(cc_in[:], part)
            nc.gpsimd.collective_compute(
                "AllReduce",
                mybir.AluOpType.add,
                replica_groups=[list(range(N_CORES))],
                ins=[cc_in.opt()],
                outs=[cc_out.opt()],
            )
            # G loads issue from the ACT sequencer: they must wait for the
            # AllReduce, and a wait on the in-order Sync stream would block
            # the pass-2 prefetch issues queued behind it.
            nc.scalar.dma_start(Gs, cc_out[0:K])
            nc.scalar.dma_start(Gq, cc_out[K:2 * K])

            # ---- scale/shift per (class, feature) ----
            mean = chain[:, 0 * F:1 * F]
            msq = chain[:, 1 * F:2 * F]
            var = chain[:, 2 * F:3 * F]
            std = chain[:, 3 * F:4 * F]
            inv = chain[:, 4 * F:5 * F]
            scal = chain[:, 5 * F:6 * F]
            shft = chain[:, 6 * F:7 * F]
            nc.vector.tensor_scalar_mul(mean, Gs, rcpt)
            nc.vector.tensor_scalar_mul(msq, Gq, rcpt)
            nc.vector.tensor_mul(var, mean, mean)
            nc.vector.tensor_sub(var, msq, var)
            nc.scalar.activation(std, var, AFT.Sqrt, bias=epst)
            nc.vector.reciprocal(inv, std)
            nc.vector.tensor_mul(scal, inv, wt)
            nc.vector.tensor_mul(shft, mean, scal)
            nc.vector.tensor_sub(shft, bt, shft)

            # ---- select per-batch scale/shift columns: [F, B_LOC] ----
            ssel_ps = psum.tile([F, B_LOC], F32)
            nc.tensor.matmul(ssel_ps[:], scal, maskTt, start=True,
                             stop=True)
            nc.vector.tensor_copy(ssel, ssel_ps[:])
            tsel_ps = psum.tile([F, B_LOC], F32)
            nc.tensor.matmul(tsel_ps[:], shft, maskTt, start=True,
                             stop=True)
            nc.vector.tensor_copy(tsel, tsel_ps[:])

            # ---- pass 2: y[:, b] = x[:, b] * ssel[:, b] + tsel[:, b] ----
            # Whole group handled by one engine (group parity): keeps the
            # ACT/DVE streams independent, no shared-tile serialization.
            # Resident groups first: their applies are ready the moment
            # ssel/tsel land, keeping stores busy while reloads stream.
            def apply_group(g, xt_tile):
                yt = yout.tile([F, GRP * L], F32)
                for i in range(GRP):
                    b = g * GRP + i
                    xs = xt_tile[:, i * L:(i + 1) * L]
                    ys = yt[:, i * L:(i + 1) * L]
                    if g % 2 == 0:
                        nc.scalar.activation(ys, xs, AFT.Identity,
                                             bias=tsel[:, b:b + 1],
                                             scale=ssel[:, b:b + 1])
                    else:
                        nc.vector.tensor_scalar(ys, xs,
                                                ssel[:, b:b + 1],
                                                tsel[:, b:b + 1],
                                                mybir.AluOpType.mult,
                                                mybir.AluOpType.add)
                nc.gpsimd.dma_start(y[:, g * GRP:(g + 1) * GRP, :], yt[:])

            # Interleave resident and streamed groups: resident applies are
            # ready the instant ssel/tsel land (stores start immediately),
            # while early streamed applies free load slots so the reload
            # stream never waits behind a block of resident-only work.
            for j in range(max(res_grp, stream_grp)):
                if j < res_grp:
                    apply_group(stream_grp + j, res_tiles[stream_grp + j])
                if j < stream_grp:
                    # First reloads reuse the freed pass-1 xin slots: deeper
                    # prefetch over the AllReduce window at no SBUF cost.
                    pool = xin if j < 5 else xin2
                    xt2 = pool.tile([F, GRP * L], F32, tag="xs")
                    # alternate load issues between the two DMA queues
                    # (Sync HWDGE / GpSimd SWDGE) to spread queue pressure
                    eng = nc.sync if j % 2 == 0 else nc.gpsimd
                    eng.dma_start(xt2[:], x[:, j * GRP:(j + 1) * GRP, :])
                    apply_group(j, xt2)

    nc.finalize()
    return nc


def _get_nc():
    global _built
    if _built is None:
        _built = _build()
    return _built


def _host_inputs(x, labels, weight, bias):
    labels = np.asarray(labels).astype(np.int64)
    counts = np.bincount(labels, minlength=K).astype(np.float64) * L
    rcp = (1.0 / np.maximum(counts, 1.0)).astype(np.float32).reshape(K, 1)
    ident = np.eye(128, dtype=np.float32)

    in_maps = []
    for c in range(N_CORES):
        lab = labels[c * B_LOC:(c + 1) * B_LOC]
        maskT = np.zeros((K, B_LOC), dtype=np.float32)
        maskT[lab, np.arange(B_LOC)] = 1.0
        mask2 = np.zeros((2 * B_LOC, 2 * K), dtype=np.float32)
        mask2[:B_LOC, :K] = maskT.T
        mask2[B_LOC:, K:] = maskT.T
        in_maps.append({
            # feature-major shard: [F, B_LOC, L]
            "x": np.ascontiguousarray(
                x[c * B_LOC:(c + 1) * B_LOC].transpose(1, 0, 2)),
            "maskT": maskT,
            "mask2": mask2,
            "ident": ident,
            "rcp_cnt": rcp,
            "epsv": np.full((K, 1), EPS, dtype=np.float32),
            "weight": np.ascontiguousarray(weight.astype(np.float32)),
            "bias": np.ascontiguousarray(bias.astype(np.float32)),
        })
    return in_maps


def run(x, labels, weight, bias, trace=False):
    nc = _get_nc()
    in_maps = _host_inputs(x, labels, weight, bias)
    res = bass_utils.run_bass_kernel_spmd(nc, in_maps, list(range(N_CORES)),
                                          trace=trace)
    out = np.concatenate(
        [res.results[c]["y"].transpose(1, 0, 2) for c in range(N_CORES)],
        axis=0)
    return out, res


def kernel(x, labels, weight, bias):
    out, _ = run(np.asarray(x, dtype=np.float32), labels,
                 np.asarray(weight, dtype=np.float32),
                 np.asarray(bias, dtype=np.float32))
    return out
